# revision 1
# baseline (speedup 1.0000x reference)
"""Trainium2 Bass kernel for nn_Blur: 4x4 FIR depthwise blur with pad (2,1).

out[n,c,i,j] = sum_{a,b} K[a,b] * x[n,c, i+1-a, j+1-b]   (zero-padded)

Strategy (8 NeuronCores, pure data parallelism over the 8192 (n,c) slices):
  - fp16 end-to-end on device (host converts): halves HBM traffic vs fp32.
    Quantization error ~5e-4 relative, far under the 2e-2 gate.
  - Main path (13 of 16 tiles): w-parity interleaved layout, partition
    p = 64*(w%2) + h; free dim packs each slice as [zero-block][32 data
    w-blocks] (33 cols), zero blocks shared between neighbours. The 16-tap
    conv is THREE PSUM-accumulated matmuls (free-dim block shifts
    d in {-1,0,+1}): lhsT_d[(jp_in,u),(jp_out,i)] = K[i-u+1, jp_out-jp_in
    +1-2d]. 12 x N=512 columns per 64-slice tile.
  - Offload path (3 tiles): separable FIR. DVE+GPSIMD do the W-conv
    (t1 = x<<0 + x<<3; t2 = x<<1 + x<<2; y2 = 3*t2 + t1) on a (member,h)
    partition layout, then ONE matmul pass applies the h-band k1/16.
    This pulls the PE (41.5us) under the fp16 DMA span (~41us) so the
    stream is DMA-bound, not PE-bound.
  - Startup: the HAM clock gate needs ~4096 cycles (3.4us) of observed PE
    activity to open (1.2 -> 2.4 GHz), and ANY idle gap resets it. Eight
    junk matmuls on a memset tile burn a full window unconditionally, so
    real matmuls start warm and never re-throttle. Tile 0 is split in
    halves so its data beats the PE to the handoff.
  - Drain: last tile stores per-PSUM-group on the ACT ring; the final
    group's copy runs on DVE alone and its store goes down the by-then-idle
    SP ring immediately.
"""

import sys
import types

import numpy as np

import concourse.bacc as bacc
import concourse.mybir as mybir
from concourse.alu_op_type import AluOpType
from concourse.tile import TileContext
from concourse.bass_utils import run_bass_kernel_spmd


def _install_ntff_hook():
    """Best-effort shim: this image's antenv lacks axon_hooks, which the
    trace=True path of run_bass_kernel_spmd imports. Harmless if unused."""
    if "antenv.axon_hooks" in sys.modules:
        return
    try:
        sys.path.insert(0, "/root/.axon_site")
        from trn_agent_boot.trn_boot import _ntff_profile_via_ctypes

        hook = _ntff_profile_via_ctypes("/opt/axon/libaxon_pjrt.so")
        mod = types.ModuleType("antenv.axon_hooks")
        mod.get_axon_ntff_profile_hook = lambda: hook
        mod.set_axon_ntff_profile_hook = lambda h: None
        sys.modules["antenv.axon_hooks"] = mod
    except Exception:
        pass


_install_ntff_hook()

N_CORES = 8
B, C, H, W = 32, 256, 64, 64
NSLICES = B * C                      # 8192
SLICES_PER_CORE = NSLICES // N_CORES  # 1024
TILE_SLICES = 64                     # slices per full SBUF tile
JB = W // 2                          # 32 w-blocks of 2 per slice
FREE = TILE_SLICES * JB              # 2048: NO padding (edge-skip matmuls)
GQ = 16                              # slices per PSUM group (N = 16*32 = 512)
GF = GQ * JB                         # free columns per group = 512
WP = W + 3                           # offload path: 2 left + 1 right zero
SG = TILE_SLICES // 2                # offload path: s-groups per member
F16 = mybir.dt.float16
F32 = mybir.dt.float32

# Separable DVE/GPS offload measured SLOWER in aggregate (engines run at
# ~75-115 G elem/s, half the spec sheet): the W-conv chain can't stay ahead
# of the PE and every slip resets the HAM clock gate. Keep it off.
OFFLOAD = ()
WARMUP_MMS = 7                       # ~one HAM window of junk matmuls

_NC_CACHE = {}


def _build_wmat(K: np.ndarray) -> np.ndarray:
    """[128, 4*128] fp16: lhsT stack [d=0, d=-1, d=+1, h-band k1/16]."""
    K = np.asarray(K, np.float32)
    wmat = np.zeros((4, 128, 128), np.float32)
    for di, d in enumerate((0, -1, 1)):
        L = wmat[di]
        for jpi in range(2):
            for jpo in range(2):
                b = jpo - jpi + 1 - 2 * d
                if not (0 <= b < 4):
                    continue
                for i in range(H):
                    for a in range(4):
                        u = i + 1 - a
                        if 0 <= u < H:
                            L[64 * jpi + u, 64 * jpo + i] += K[a, b]
    # h-band for the separable path: lhsT[u+64m, i+64m] = k1[i-u+1]/16
    k1 = np.array([1.0, 3.0, 3.0, 1.0], np.float32) / 16.0
    T = np.zeros((H, H), np.float32)
    for i in range(H):
        for a in range(4):
            u = i + 1 - a
            if 0 <= u < H:
                T[u, i] += k1[a]
    wmat[3, :H, :H] = T
    wmat[3, H:, H:] = T
    # [d, k, m] -> [k, (d m)] so the DMA is one contiguous run per partition
    return np.ascontiguousarray(
        wmat.transpose(1, 0, 2).reshape(128, 4 * 128)
    ).astype(np.float16)


def _build_nc(slices_per_core: int = SLICES_PER_CORE):
    ntiles = slices_per_core // TILE_SLICES
    nc = bacc.Bacc("TRN2", target_bir_lowering=False, debug=False)
    x = nc.dram_tensor(
        "x", [ntiles, 128, FREE], F16, kind="ExternalInput"
    ).ap()
    xo = (
        nc.dram_tensor(
            "xo", [len(OFFLOAD), 128, SG * WP], F16, kind="ExternalInput"
        ).ap()
        if OFFLOAD
        else None
    )
    wm = nc.dram_tensor("w", [128, 4 * 128], F16, kind="ExternalInput").ap()
    y = nc.dram_tensor(
        "y", [ntiles, 128, TILE_SLICES * JB], F16, kind="ExternalOutput"
    ).ap()
    # sink for the PE warm-up matmuls (kept alive so DCE can't drop them)
    warm_out = nc.dram_tensor("warm", [128, 4], F32, kind="ExternalOutput").ap()

    # main-path chunk = (dram tile, first group, n groups); tile 0 is
    # halved: the first 262KB landing starts the real stream ~1us after
    # warmup ends, and the second half's ~1.6us completion receipt hides
    # under the first half's six matmuls. (A finer 1+1+2 split was
    # measured: the 4th load's receipt then lands AFTER the PE needs it.)
    chunks = [(0, 0, 2), (0, 2, 2)]
    chunks += [(t, 0, 4) for t in range(1, ntiles) if t not in OFFLOAD]
    last = len(chunks) - 1

    with TileContext(nc) as tc:
        with (
            tc.tile_pool(name="wpool", bufs=1) as wpool,
            tc.tile_pool(name="xpool", bufs=8) as xpool,
            tc.tile_pool(name="vpool", bufs=4) as vpool,
            tc.tile_pool(name="opool", bufs=6) as opool,
            tc.tile_pool(name="pspool", bufs=8, space="PSUM") as pspool,
        ):
            # weight tile: its DMA is issued from inside the chunk loop (2nd
            # slot on the SP ring) -- weights are only needed after warmup,
            # so tile 0's first group goes down the ring first.
            wsb = wpool.tile([128, 4, 128], F16, name="wsb")

            # HAM warm-up: burn one full throttle window on junk matmuls
            # (no DMA dependency) so every real matmul runs at 2.4 GHz.
            # The memset runs on the otherwise-idle Pool engine, whose body
            # starts a shade earlier than DVE's.
            wjunk = wpool.tile([128, 512], F16, name="wjunk")
            nc.gpsimd.memset(wjunk[:], 0.0)
            wscratch = wpool.tile([128, 4], F32, name="wscratch")
            wps = pspool.tile([128, 512], F32, name="wps", tag="ps")
            for r in range(WARMUP_MMS):
                nc.tensor.matmul(
                    wps[:],
                    wjunk[:, 0:128],
                    wjunk[:],
                    start=(r == 0),
                    stop=(r == WARMUP_MMS - 1),
                )
            nc.vector.tensor_copy(wscratch[:], wps[:, 0:4])
            nc.scalar.dma_start(warm_out, wscratch[:])

            oi = {t: i for i, t in enumerate(OFFLOAD)}
            ncopy = 0

            def offload_tile(t):
                """Separable path: W-conv on DVE/GPS, one h-band PE pass."""
                xt = xpool.tile([128, SG, WP], F16, name="xof")
                nc.sync.dma_start(xt[:], xo[oi[t]])
                t1 = vpool.tile([128, SG, W], F16, name="t1")
                t2 = vpool.tile([128, SG, W], F16, name="t2")
                y2 = vpool.tile([128, SG, W], F16, name="y2")
                # both adds on GPSIMD (SBUF-only ops are Pool-legal); the
                # fused 3*t2+t1 is DVE-only (TensorScalarPtr not on Pool)
                nc.gpsimd.tensor_tensor(
                    t1[:], xt[:, :, 0:W], xt[:, :, 3 : 3 + W], AluOpType.add
                )
                nc.gpsimd.tensor_tensor(
                    t2[:], xt[:, :, 1 : 1 + W], xt[:, :, 2 : 2 + W],
                    AluOpType.add,
                )
                nc.vector.scalar_tensor_tensor(
                    y2[:], t2[:], 3.0, t1[:],
                    op0=AluOpType.mult, op1=AluOpType.add,
                )
                ot = opool.tile([128, SG, W], F16, name="ot")
                for q in range(4):
                    ps = pspool.tile([128, GQ * JB], F32, name="ps")
                    nc.tensor.matmul(
                        ps[:], wsb[:, 3, :], y2[:, 8 * q : 8 * (q + 1), :],
                        start=True, stop=True,
                    )
                    dst = ot[:, 8 * q : 8 * (q + 1), :]
                    if q % 2 == 0:
                        nc.scalar.copy(dst, ps[:])
                    else:
                        nc.vector.tensor_copy(dst, ps[:])
                nc.scalar.dma_start(y[t], ot[:])

            for ci, (dt, g0, ng) in enumerate(chunks):
                xt = xpool.tile([128, ng * GQ, JB], F16, name="xt")
                nc.sync.dma_start(
                    xt[:], x[dt][:, g0 * GF : (g0 + ng) * GF]
                )
                if ci == 0:
                    # weights ride the SP ring second: ready ~when warmup ends
                    nc.sync.dma_start(wsb[:], wm)

                ot = opool.tile([128, ng * GQ, JB], F16, name="ot")
                pss = [
                    pspool.tile([128, GQ, JB], F32, name="ps")
                    for _ in range(ng)
                ]
                # d-outer: one stationary load per pass. No padding: the
                # d=-1 pass would read only zeros for jb=0 and d=+1 only
                # zeros for jb=31, so those output columns are simply
                # SKIPPED (496-column matmuls) -- 2% less PE work and the
                # DMA carries pure data.
                for di, d in enumerate((0, -1, 1)):
                    for q in range(ng):
                        s0, s1 = GQ * q, GQ * (q + 1)
                        if d == 0:
                            rhs, dst = xt[:, s0:s1, :], pss[q][:]
                        elif d == -1:
                            rhs = xt[:, s0:s1, 0 : JB - 1]
                            dst = pss[q][:, :, 1:JB]
                        else:
                            rhs = xt[:, s0:s1, 1:JB]
                            dst = pss[q][:, :, 0 : JB - 1]
                        nc.tensor.matmul(
                            dst,
                            wsb[:, di, :],
                            rhs,
                            start=(di == 0),
                            stop=(di == 2),
                        )
                for q in range(ng):
                    dst = ot[:, GQ * q : GQ * (q + 1), :]
                    ylo = (g0 + q) * GQ * JB
                    if ci == last and q == ng - 1:
                        # final group: DVE-only copy, store on the idle SP
                        # ring the moment it lands -- shortest tail
                        nc.vector.tensor_copy(dst, pss[q][:])
                        nc.sync.dma_start(
                            y[dt][:, ylo : ylo + GQ * JB], dst
                        )
                        continue
                    # alternate copy engine: DVE and ACT share the load
                    if ncopy % 2 == 0:
                        nc.vector.tensor_copy(dst, pss[q][:])
                    else:
                        nc.scalar.copy(dst, pss[q][:])
                    ncopy += 1
                    if ci == last:
                        # alternate rings so the drain issues in parallel
                        eng = nc.scalar if q % 2 == 0 else nc.sync
                        eng.dma_start(y[dt][:, ylo : ylo + GQ * JB], dst)
                if ci != last:
                    ylo = g0 * GQ * JB
                    nc.scalar.dma_start(
                        y[dt][:, ylo : ylo + ng * GQ * JB], ot[:]
                    )
                # interleave offloaded tiles after their preceding chunk
                nt = dt + 1
                if g0 + ng == 4 and nt in oi and nt < ntiles:
                    offload_tile(nt)

    nc.compile()
    return nc


def get_nc(slices_per_core: int = SLICES_PER_CORE):
    if slices_per_core not in _NC_CACHE:
        _NC_CACHE[slices_per_core] = _build_nc(slices_per_core)
    return _NC_CACHE[slices_per_core]


def _pack_input(xs: np.ndarray):
    """[S, H, W] fp16 -> main tiles [S/64, 128, FREE] + offload tiles."""
    s = xs.shape[0]
    ntiles = s // TILE_SLICES
    v = np.empty((ntiles, 2, H, TILE_SLICES, JB), np.float16)
    xt = xs.reshape(ntiles, TILE_SLICES, H, W)
    v[:, 0] = xt[:, :, :, 0::2].transpose(0, 2, 1, 3)
    v[:, 1] = xt[:, :, :, 1::2].transpose(0, 2, 1, 3)
    xmain = np.ascontiguousarray(v.reshape(ntiles, 128, FREE))
    if not OFFLOAD:
        return xmain, None
    # offload tiles: partition (m, h), free (sg, w) with w zero-padded to 67
    xofs = np.zeros((len(OFFLOAD), 128, SG * WP), np.float16)
    for i, t in enumerate(OFFLOAD):
        xp = np.zeros((TILE_SLICES, H, WP), np.float16)
        xp[:, :, 2 : 2 + W] = xt[t]
        # (sg, m, h, w) -> (m, h, sg, w)
        xofs[i] = (
            xp.reshape(SG, 2, H, WP)
            .transpose(1, 2, 0, 3)
            .reshape(128, SG * WP)
        )
    return xmain, xofs


def _unpack_output(yp: np.ndarray) -> np.ndarray:
    """[S/64, 128, 64*JB] fp16 -> [S, H, W] fp16 (mixed per-tile layouts)."""
    ntiles = yp.shape[0]
    out = np.empty((ntiles, TILE_SLICES, H, W), np.float16)
    # main path: [jp, i, s, jb]
    v = yp.reshape(ntiles, 2, H, TILE_SLICES, JB)
    out[:, :, :, 0::2] = v[:, 0].transpose(0, 2, 1, 3)
    out[:, :, :, 1::2] = v[:, 1].transpose(0, 2, 1, 3)
    # offload path: [m, i, sg, w]
    for t in OFFLOAD:
        if t < ntiles:
            vo = yp[t].reshape(2, H, SG, W)
            out[t] = vo.transpose(2, 0, 1, 3).reshape(TILE_SLICES, H, W)
    return out.reshape(ntiles * TILE_SLICES, H, W)


def kernel(x: np.ndarray, kernel: np.ndarray, _trace: bool = False, **_tkw):
    xh = np.asarray(x).astype(np.float16)
    wmat = _build_wmat(kernel)
    b, c, h, w = x.shape
    xs = xh.reshape(b * c, h, w)
    spc = (b * c) // N_CORES
    nc = get_nc(spc)
    in_maps = []
    for k in range(N_CORES):
        xmain, xofs = _pack_input(xs[k * spc : (k + 1) * spc])
        m = {"x": xmain, "w": wmat}
        if xofs is not None:
            m["xo"] = xofs
        in_maps.append(m)
    res = run_bass_kernel_spmd(
        nc, in_maps, list(range(N_CORES)), trace=_trace, **_tkw
    )
    out = np.concatenate(
        [_unpack_output(res.results[k]["y"]) for k in range(N_CORES)], axis=0
    )
    result = out.reshape(b, c, h, w).astype(np.float32)
    if _trace:
        return result, res
    return result

